# revision 7
# baseline (speedup 1.0000x reference)
"""Trainium2 Bass kernel for CrowdCountingLoss (debiased unbalanced Sinkhorn).

Math: the 4096x4096 cost matrix C over the 64x64 grid is separable
(C = 0.5 dx^2 + 0.5 dy^2), so the Gibbs kernel factorizes as a Kronecker
product: exp(-C/eps) = K (x) K with K[a,b] = exp(-0.5 (a-b)^2/eps), a 64x64
matrix. Each softmin's logsumexp row-reduction sum_j exp(h_j - C_ij/eps)
becomes S = K @ reshape(exp(h),64,64) @ K -- two 64^3 matmuls on the PE
instead of a 16.7M-element sweep (|h| < ~10 over the whole annealing
schedule, so no max-shift is needed inside the lse; S never under/overflows).

On the PE (out = lhsT.T @ rhs): A = mm(lhsT=W, rhs=K) = W.T K, then
S = mm(lhsT=A, rhs=K) = K W K (K symmetric) -- no transposes needed.

Structure: two independent pipelines ("chains") run per annealing step so the
Tile scheduler can overlap engines:
  chain P: the coupled pair {f_ba, g_ab} (each updates from the other's
           softmin -- realized by writing the second matmul's output into the
           partner's slot, a free "swap");
  chain S: the self-coupled {f_aa, g_bb}.
Blocks are stacked on partition halves ([128,64] tiles). Potentials are kept
scaled as R_k = 2^k P_k, which turns the averaged update
P_{k+1} = (P_k + c_k ln S_k)/2 into a single fused DVE op
R_{k+1} = R_k + (2^k c_k) L_k (power-of-two scaling is exact in fp32), and
h_{k+1} = X + P_{k+1}/eps_{k+1} into h = (c_k/(2 eps'))*L + D with
D = X + R_k/(2^{k+1} eps') computed one full step ahead of its use, keeping
the per-step critical path to: ln -> h -> exp -> mm1 x2 -> copy -> mm2 x2.

The 60 annealing steps are fully unrolled; the whole state is <1 MB, so all
8 cores run the computation redundantly and core 0's output is returned.
Matmuls run in bf16 (fp32 accumulate): validated to move the loss by ~1e-4
absolute (~3e-6 relative), the same order as the fp32 reference's own
distance from an fp64 evaluation.
"""

import json

import numpy as np

import concourse.bass as bass
import concourse.bass2jax as bass2jax
import concourse.bass_utils as bass_utils
import concourse.mybir as mybir
import concourse.tile as tile

# ---------------------------------------------------------------------------
# Workaround: the walrus build in this container supports only ONE semaphore
# wait per instruction ("Too many sync wait commands" in setupSyncWait).
# Split any multi-wait instruction into single-wait NoOp prefixes on the same
# engine (all waits still complete before the original instruction issues).
# ---------------------------------------------------------------------------
_orig_compile_bir_kernel = bass_utils.compile_bir_kernel


def _split_multiwait_bir(bir_json: bytes) -> bytes:
    m = json.loads(bir_json)
    changed = False
    for fn in m.get("functions", []):
        for bb in fn.get("blocks", []):
            out = []
            for inst in bb.get("instructions", []):
                si = inst.get("sync_info")
                if si:
                    waits = si.get("on_wait") or []
                    if len(waits) > 1:
                        for k, w in enumerate(waits[:-1]):
                            out.append({
                                "debug": inst.get("debug", 0),
                                "engine": inst["engine"],
                                "ins": [],
                                "name": f"{inst['name']}_mw{k}",
                                "opcode": "NoOp",
                                "outs": [],
                                "sync_info": {"on_update": [], "on_wait": [w]},
                            })
                        si["on_wait"] = [waits[-1]]
                        changed = True
                out.append(inst)
            bb["instructions"] = out
    if not changed:
        return bir_json
    return json.dumps(m).encode()


def _patched_compile_bir_kernel(bir_json, tmpdir, neff_name="file.neff"):
    return _orig_compile_bir_kernel(_split_multiwait_bir(bir_json), tmpdir,
                                    neff_name)


bass_utils.compile_bir_kernel = _patched_compile_bir_kernel
bass2jax.compile_bir_kernel = _patched_compile_bir_kernel

# ---------------------------------------------------------------------------
# Problem constants (CrowdCountingLoss init kwargs; 64x64 grid)
# ---------------------------------------------------------------------------
ALPHA = 0.1
BLUR = 0.2
SCALING = 0.9
REACH = 0.1
RHO = REACH**2          # 0.01
EPS_FIN = BLUR**2       # 0.04
N_CORES = 8
MM_DTYPE = "bf16"       # "f32" | "bf16"

F32 = mybir.dt.float32
BF16 = mybir.dt.bfloat16
AF = mybir.ActivationFunctionType
ALU = mybir.AluOpType
CH = ("P", "S")


def _eps_schedule() -> np.ndarray:
    diam = float(np.sqrt(63.0**2 + 63.0**2))
    sched = (
        [diam**2]
        + list(np.exp(np.arange(2 * np.log(diam), 2 * np.log(BLUR),
                                2 * np.log(SCALING))))
        + [BLUR**2]
    )
    return np.asarray(sched, dtype=np.float32)


def _k_stack(eps_arr: np.ndarray) -> np.ndarray:
    idx = np.arange(64, dtype=np.float64)
    d2 = (idx[:, None] - idx[None, :]) ** 2
    ks = [np.exp(-0.5 * d2 / np.float64(e)).astype(np.float32)
          for e in eps_arr]
    return np.ascontiguousarray(np.concatenate(ks, axis=1))


def _build(mm_dtype=MM_DTYPE):
    eps_arr = _eps_schedule()
    n_eps = len(eps_arr)
    kstack = _k_stack(eps_arr)
    kstack2 = np.concatenate([kstack, kstack], axis=0)  # [128, n_eps*64]
    MMD = F32 if mm_dtype == "f32" else BF16
    if mm_dtype == "bf16":
        import ml_dtypes
        kstack2 = kstack2.astype(ml_dtypes.bfloat16)
    kstack2 = np.ascontiguousarray(kstack2)

    nc = bass.Bass("TRN2", target_bir_lowering=False, debug=False,
                   num_devices=N_CORES)
    pred_d = nc.dram_tensor("pred_map", [64, 64], F32,
                            kind="ExternalInput").ap()
    gt_d = nc.dram_tensor("gt_grid", [64, 64], F32, kind="ExternalInput").ap()
    ks_d = nc.dram_tensor("kstack", [128, n_eps * 64], MMD,
                          kind="ExternalInput").ap()
    loss_d = nc.dram_tensor("loss", [1, 1], F32, kind="ExternalOutput").ap()

    eps = [float(e) for e in eps_arr]
    lam = [RHO / (RHO + e) for e in eps]
    c = [-lam[k] * eps[k] for k in range(n_eps)]
    eps_next = eps[1:] + [EPS_FIN]

    with tile.TileContext(nc) as tc:
        with (
            tc.tile_pool(name="singles", bufs=1) as singles,
            tc.tile_pool(name="work", bufs=3) as work,
            tc.tile_pool(name="psum", bufs=1, space="PSUM") as psp,
        ):
            KS = singles.tile([128, n_eps * 64], MMD)
            nc.sync.dma_start(out=KS, in_=ks_d)
            ABs = singles.tile([128, 64], F32)  # [a; b] stacked (a=pred, b=gt)
            nc.sync.dma_start(out=ABs[0:64, :], in_=pred_d)
            nc.sync.dma_start(out=ABs[64:128, :], in_=gt_d)
            BAs = singles.tile([128, 64], F32)  # [b; a]
            nc.sync.dma_start(out=BAs[0:64, :], in_=gt_d)
            nc.sync.dma_start(out=BAs[64:128, :], in_=pred_d)

            XP = singles.tile([128, 64], F32)  # [ln b; ln a]
            XS = singles.tile([128, 64], F32)  # [ln a; ln b]
            nc.scalar.activation(out=XP, in_=BAs, func=AF.Ln)
            nc.scalar.activation(out=XS, in_=ABs, func=AF.Ln)
            X = {"P": XP, "S": XS}

            ones = singles.tile([128, 1], F32)
            nc.vector.memset(ones, 1.0)
            wv = singles.tile([1, 3], F32)  # [spatial, density, count] weights
            w_fin = RHO + EPS_FIN / 2
            nc.vector.memset(wv[:, 0:1], ALPHA * w_fin)
            nc.vector.memset(wv[:, 1:2], 1.0 / 4096.0)
            nc.vector.memset(wv[:, 2:3], 1.0)

            R = {}
            for ch in CH:
                for i in range(2):
                    R[ch, i] = singles.tile([128, 64], F32, tag=f"R{ch}{i}",
                                            name=f"R{ch}{i}")

            def kb(i_eps, half):
                return KS[half * 64: half * 64 + 64,
                          i_eps * 64: (i_eps + 1) * 64]

            def do_mm1(ch, W4, i_eps):
                ps1 = psp.tile([128, 64], F32, tag=f"ps1{ch}",
                               name=f"ps1{ch}")
                nc.tensor.matmul(ps1[0:64, :], W4[0:64, :], kb(i_eps, 0),
                                 start=True, stop=True)
                nc.tensor.matmul(ps1[64:128, :], W4[64:128, :], kb(i_eps, 1),
                                 start=True, stop=True)
                return ps1

            def do_cp(ch, ps1):
                A2 = work.tile([128, 64], MMD, tag=f"A{ch}", name=f"A{ch}")
                nc.vector.tensor_copy(out=A2, in_=ps1)
                return A2

            def do_mm2(ch, A2, i_eps, swapped):
                ps2 = psp.tile([128, 64], F32, tag=f"ps2{ch}",
                               name=f"ps2{ch}")
                top_out = ps2[64:128, :] if swapped else ps2[0:64, :]
                bot_out = ps2[0:64, :] if swapped else ps2[64:128, :]
                nc.tensor.matmul(top_out, A2[0:64, :], kb(i_eps, 0),
                                 start=True, stop=True)
                nc.tensor.matmul(bot_out, A2[64:128, :], kb(i_eps, 1),
                                 start=True, stop=True)
                return ps2

            def sweep_all(W4, i_eps, last):
                ps1 = {ch: do_mm1(ch, W4[ch], i_eps) for ch in CH}
                A2 = {ch: do_cp(ch, ps1[ch]) for ch in CH}
                return {ch: do_mm2(ch, A2[ch], i_eps,
                                   swapped=(ch == "P" and not last))
                        for ch in CH}

            # ---- init (W = raw weights; exp(ln x) == x skipped exactly) --
            Ls = {}
            if MMD == F32:
                W0 = {"P": BAs, "S": ABs}
            else:
                W0 = {}
                for ch, srct in (("P", BAs), ("S", ABs)):
                    t = singles.tile([128, 64], MMD, name=f"W0{ch}")
                    nc.vector.tensor_copy(out=t, in_=srct)
                    W0[ch] = t
            ps2i = sweep_all(W0, 0, last=False)
            for ch in CH:
                L = work.tile([128, 64], F32, tag=f"L{ch}", name=f"L{ch}i")
                nc.scalar.activation(out=L, in_=ps2i[ch], func=AF.Ln)
                Ls[ch] = L
                nc.vector.tensor_scalar_mul(R[ch, 0], L, c[0])

            hc = {ch: c[0] / eps[0] for ch in CH}
            D_prev = {"P": XP, "S": XS}
            cur = 0
            pending_R = None

            # ---- 60 annealing steps + final extrapolation (unrolled) -----
            for k in range(n_eps + 1):
                last = k == n_eps
                i_eps = min(k, n_eps - 1)
                h = {}
                for ch in CH:
                    h[ch] = work.tile([128, 64], F32, tag=f"h{ch}",
                                      name=f"h{ch}")
                    nc.vector.scalar_tensor_tensor(
                        out=h[ch], in0=Ls[ch], scalar=hc[ch], in1=D_prev[ch],
                        op0=ALU.mult, op1=ALU.add)
                # deferred R update (off the critical path, after the h's)
                if pending_R is not None:
                    pk, pL = pending_R
                    s_pk = float(2.0 ** pk)
                    for ch in CH:
                        nc.vector.scalar_tensor_tensor(
                            out=R[ch, 1 - cur], in0=pL[ch],
                            scalar=s_pk * c[pk], in1=R[ch, cur],
                            op0=ALU.mult, op1=ALU.add)
                    cur = 1 - cur
                    pending_R = None
                W4 = {}
                for ch in CH:
                    W4[ch] = work.tile([128, 64], MMD, tag=f"W{ch}",
                                       name=f"W{ch}")
                    nc.scalar.activation(out=W4[ch], in_=h[ch], func=AF.Exp)
                ps2 = sweep_all(W4, i_eps, last)
                newL = {}
                for ch in CH:
                    L = work.tile([128, 64], F32, tag=f"L{ch}", name=f"L{ch}")
                    nc.scalar.activation(out=L, in_=ps2[ch], func=AF.Ln)
                    newL[ch] = L
                if not last:
                    s_k = float(2.0 ** k)
                    for ch in CH:
                        # D_k from the OLD R_k -> ready one step early
                        D = work.tile([128, 64], F32, tag=f"D{ch}",
                                      name=f"D{ch}")
                        nc.vector.scalar_tensor_tensor(
                            out=D, in0=R[ch, cur],
                            scalar=1.0 / (2.0 * s_k * eps_next[k]), in1=X[ch],
                            op0=ALU.mult, op1=ALU.add)
                        D_prev[ch] = D
                        hc[ch] = c[k] / (2.0 * eps_next[k])
                    pending_R = (k, newL)
                Ls = newL

            # ---- loss assembly ------------------------------------------
            # final L (unswapped): L_P=[ln S_fba; ln S_gab],
            #                      L_S=[ln S_faa; ln S_gbb]
            kappa = lam[-1] * EPS_FIN / RHO
            E = {}
            for ch in CH:
                Et = work.tile([128, 64], F32, tag=f"E{ch}", name=f"E{ch}")
                nc.scalar.activation(out=Et, in_=Ls[ch], func=AF.Exp,
                                     scale=kappa)
                E[ch] = Et
            cols = singles.tile([128, 3], F32)
            nc.vector.memset(cols, 0.0)
            junk = work.tile([128, 64], F32, tag="junk", name="junk")
            junk2 = work.tile([64, 64], F32, tag="junk2", name="junk2")
            # spatial: [E_faa-E_fba ; E_gbb-E_gab] dot [a; b]
            dsp = work.tile([128, 64], F32, tag="dsp", name="dsp")
            nc.vector.tensor_sub(dsp, E["S"], E["P"])
            nc.vector.scalar_tensor_tensor(
                out=junk, in0=dsp, scalar=1.0, in1=ABs,
                op0=ALU.mult, op1=ALU.mult, accum_out=cols[:, 0:1])
            d_ab = work.tile([64, 64], F32, tag="d_ab", name="d_ab")
            nc.vector.tensor_sub(d_ab, ABs[0:64, :], BAs[0:64, :])
            nc.scalar.activation(out=junk2, in_=d_ab, func=AF.Square,
                                 accum_out=cols[0:64, 1:2])
            nc.vector.reduce_sum(cols[0:64, 2:3], d_ab,
                                 axis=mybir.AxisListType.X)
            ps3 = psp.tile([1, 3], F32, tag="ps3", name="ps3")
            nc.tensor.matmul(ps3, ones, cols, start=True, stop=True)
            s13 = singles.tile([1, 3], F32)
            nc.vector.tensor_copy(out=s13, in_=ps3)
            nc.scalar.activation(out=s13[:, 2:3], in_=s13[:, 2:3], func=AF.Abs)
            res = singles.tile([1, 1], F32)
            junk3 = singles.tile([1, 3], F32)
            nc.vector.scalar_tensor_tensor(
                out=junk3, in0=s13, scalar=1.0, in1=wv,
                op0=ALU.mult, op1=ALU.mult, accum_out=res)
            nc.sync.dma_start(out=loss_d, in_=res)

    return nc, kstack2


_CACHE: dict = {}


def kernel(pred_map: np.ndarray, gt_map: np.ndarray,
           gt_blur_map: np.ndarray = None, **_unused) -> np.ndarray:
    if "nc" not in _CACHE:
        _CACHE["nc"], _CACHE["kstack"] = _build()
    nc, kstack = _CACHE["nc"], _CACHE["kstack"]
    in_map = {
        "pred_map": np.ascontiguousarray(pred_map, dtype=np.float32),
        "gt_grid": np.ascontiguousarray(
            np.asarray(gt_map, dtype=np.float32).reshape(64, 64)),
        "kstack": kstack,
    }
    out = bass_utils.run_bass_kernel_spmd(
        nc, [in_map] * N_CORES, core_ids=list(range(N_CORES)))
    return np.float32(out.results[0]["loss"].reshape(())[()])
